# revision 11
# baseline (speedup 1.0000x reference)
"""Trainium2 Bass kernel for nn_PatchMMConvolution.

Computes a shared-weight 3x3 conv (stride 1, pad 1) over x[B=2, P=18, Cin=64,
H=128, W=128] with weight[Cout=128, Cin=64, 3, 3] + bias, i.e. conv2d on
36 images, returning [2, 18, 128, 128, 128] float32.

Strategy (8 NeuronCores, SPMD single program):
  - 36 images are split into 16 "streams" of 288 output rows each
    (2 full images + one quarter-image per stream). Each core runs two
    streams: stream A in SBUF partitions 0-63, stream B in partitions 64-127
    (Cin=64 channels live on partitions).
  - Host pre-pads each stream into a "slab" [64, 294, 130]: three vertically
    concatenated zero-padded segments (130+130+34 rows, W padded to 130).
  - Conv is 9 shifted matmuls accumulating in PSUM: for each tap (kh,kw),
    lhsT = weight[kh,kw] as [Cin=64, Cout=128], rhs = shifted input window
    [64, 4 rows x 128 cols] (N=512). K=64 matmuls for streams A and B use
    PE row-groups 0-1 and 2-3 concurrently (tile_position derived from the
    base partition), so the two streams overlap on the PE array.
  - Bias is added during the PSUM->SBUF evacuation on the Vector engine.
"""

import numpy as np

import concourse.bass as bass
import concourse.mybir as mybir
import concourse.tile as tile
from concourse import bacc
from concourse._compat import get_trn_type
from concourse.bass_utils import run_bass_kernel_spmd

B, PP, CIN, H, W = 2, 18, 64, 128, 128
COUT = 128
NIMG = B * PP  # 36
NCORES = 8
NSTREAM = 16
WP = W + 2  # 130 padded width
RSLAB = 294  # 130 + 130 + 34 slab rows per stream
ROWS_PER_STREAM = 288
# (slab_row_base, out_row_base, out_rows) per segment
SEGS = [(0, 0, 128), (130, 128, 128), (260, 256, 32)]
CHUNK_OUT_ROWS = 32  # output rows per input chunk
CHUNK_ROWS = CHUNK_OUT_ROWS + 2  # 34 input rows per chunk
TILE_OUT_ROWS = 4  # output rows per matmul tile (4*128 = 512 = one PSUM bank)

DT = mybir.dt.float32r  # matmul input dtype (fp32 data, fast PE path)
ACC = mybir.dt.float32

# Benchmark knob: repeat the whole kernel body KERNEL_REPS times inside a
# hardware loop (used to isolate device exec time from dispatch overhead).
KERNEL_REPS = 1
PSUM_BUFS = 4  # buffers per psum tag (psA/psB); 2 tags -> 2*PSUM_BUFS banks
IN_BUFS = 4  # input chunk double-buffering depth
OUT_BUFS = 4  # output tile buffering depth

_PROGRAM = None


def _build_program():
    nc = bacc.Bacc(get_trn_type() or "TRN2", target_bir_lowering=False)
    xs = nc.dram_tensor("xs", [128, RSLAB, WP], DT, kind="ExternalInput")
    wd = nc.dram_tensor("wt", [128, 9, COUT], DT, kind="ExternalInput")
    bd = nc.dram_tensor("bias", [COUT, 1], ACC, kind="ExternalInput")
    od = nc.dram_tensor(
        "out", [COUT, 2, ROWS_PER_STREAM, W], ACC, kind="ExternalOutput"
    )

    chunks = []
    for sb, ob, nr in SEGS:
        for j in range(nr // CHUNK_OUT_ROWS):
            chunks.append((sb + CHUNK_OUT_ROWS * j, ob + CHUNK_OUT_ROWS * j))

    with tile.TileContext(nc) as tc:
        with (
            tc.tile_pool(name="const", bufs=1) as cpool,
            tc.tile_pool(name="inp", bufs=IN_BUFS) as ipool,
            tc.tile_pool(name="outp", bufs=OUT_BUFS) as opool,
            tc.tile_pool(name="ps", bufs=PSUM_BUFS, space="PSUM") as pspool,
        ):
            w_sb = cpool.tile([128, 9, COUT], DT)
            nc.sync.dma_start(w_sb[:], wd[:])
            b_sb = cpool.tile([COUT, 1], ACC)
            nc.sync.dma_start(b_sb[:], bd[:])

            def emit_body():
                for srow, orow in chunks:
                    ch = ipool.tile([128, CHUNK_ROWS, WP], DT, tag="chunk")
                    nc.sync.dma_start(ch[:], xs[:, srow : srow + CHUNK_ROWS, :])
                    for i in range(CHUNK_OUT_ROWS // TILE_OUT_ROWS):
                        psa = pspool.tile([128, TILE_OUT_ROWS, W], ACC, tag="psA")
                        psb = pspool.tile([128, TILE_OUT_ROWS, W], ACC, tag="psB")
                        r0 = TILE_OUT_ROWS * i
                        for tap in range(9):
                            kh, kw = divmod(tap, 3)
                            first, last = tap == 0, tap == 8
                            nc.tensor.matmul(
                                psa[:],
                                w_sb[0:64, tap],
                                ch[0:64, r0 + kh : r0 + kh + TILE_OUT_ROWS, kw : kw + W],
                                start=first,
                                stop=last,
                            )
                            nc.tensor.matmul(
                                psb[:],
                                w_sb[64:128, tap],
                                ch[64:128, r0 + kh : r0 + kh + TILE_OUT_ROWS, kw : kw + W],
                                start=first,
                                stop=last,
                            )
                        oa = opool.tile([128, TILE_OUT_ROWS, W], ACC, tag="oA")
                        obt = opool.tile([128, TILE_OUT_ROWS, W], ACC, tag="oB")
                        nc.vector.tensor_scalar_add(oa[:], psa[:], b_sb[:])
                        nc.vector.tensor_scalar_add(obt[:], psb[:], b_sb[:])
                        orr = orow + r0
                        nc.sync.dma_start(
                            od[:, 0, orr : orr + TILE_OUT_ROWS, :], oa[:]
                        )
                        nc.sync.dma_start(
                            od[:, 1, orr : orr + TILE_OUT_ROWS, :], obt[:]
                        )

            if KERNEL_REPS > 1:
                with tc.For_i(0, KERNEL_REPS, 1) as _i:
                    emit_body()
            else:
                emit_body()
    nc.finalize()
    return nc


def _get_program():
    global _PROGRAM
    if _PROGRAM is None:
        _PROGRAM = _build_program()
    return _PROGRAM


def _stream_parts(s):
    """Stream s covers full images 2s, 2s+1 and quarter (s%4) of image 32+(s//4)...
    returns (img0, img1, img_q, q) with quarter rows [32q, 32q+32)."""
    img_q = 32 + (s % 4)
    q = s // 4
    return 2 * s, 2 * s + 1, img_q, q


def _make_slab(X, s):
    """Build padded slab [CIN, RSLAB, WP] for stream s from X [NIMG,CIN,H,W]."""
    i0, i1, iq, q = _stream_parts(s)
    sl = np.zeros((CIN, RSLAB, WP), np.float32)
    sl[:, 1 : H + 1, 1 : W + 1] = X[i0]
    sl[:, 131 : 131 + H, 1 : W + 1] = X[i1]
    r0 = 32 * q
    lo, hi = max(r0 - 1, 0), min(r0 + 33, H)
    d0 = 260 + (lo - (r0 - 1))
    sl[:, d0 : d0 + (hi - lo), 1 : W + 1] = X[iq, :, lo:hi]
    return sl


def kernel(x, weight, bias):
    x = np.ascontiguousarray(np.asarray(x), dtype=np.float32)
    weight = np.ascontiguousarray(np.asarray(weight), dtype=np.float32)
    bias = np.ascontiguousarray(np.asarray(bias), dtype=np.float32)
    X = x.reshape(NIMG, CIN, H, W)

    wt = np.ascontiguousarray(weight.transpose(1, 2, 3, 0).reshape(CIN, 9, COUT))
    wt2 = np.ascontiguousarray(np.concatenate([wt, wt], axis=0))  # [128, 9, COUT]
    bb = np.ascontiguousarray(bias.reshape(COUT, 1))

    in_maps = []
    for c in range(NCORES):
        xs = np.concatenate([_make_slab(X, 2 * c), _make_slab(X, 2 * c + 1)], axis=0)
        in_maps.append({"xs": np.ascontiguousarray(xs), "wt": wt2, "bias": bb})

    nc = _get_program()
    res = run_bass_kernel_spmd(nc, in_maps, core_ids=list(range(NCORES)))

    Y = np.empty((NIMG, COUT, H, W), np.float32)
    for c in range(NCORES):
        o = res.results[c]["out"]  # [COUT, 2, 288, W]
        for half in (0, 1):
            s = 2 * c + half
            i0, i1, iq, q = _stream_parts(s)
            oo = o[:, half]
            Y[i0] = oo[:, 0:H]
            Y[i1] = oo[:, H : 2 * H]
            Y[iq, :, 32 * q : 32 * q + 32, :] = oo[:, 2 * H : 2 * H + 32]
    return Y.reshape(B, PP, COUT, H, W)
